# revision 1
# baseline (speedup 1.0000x reference)
"""CRF loss kernel for Trainium2 (8 NeuronCores, data-parallel over batch).

Math (faithful to the reference):
  loss = (forscore - tg_energy) / B
  tg_energy = B*trans[0,START] + sum_bt scores[b,t,0] + sum_bt trans[0, gold[b,t]]
    (the reference's torch.gather-on-flattened-(L*L) quirk reduces to row 0)
  forscore = sum_b fs_T[b, END], where fs is the standard CRF forward recurrence
    fs_{t+1}[j] = logsumexp_i(fs_t[i] + scores[t,i] + trans[i,j]), fs_0 = trans[START,:]

Device algorithm, linear space with E = exp(trans) (bf16 matmuls, f32 PSUM):
  forward half  (t = 0..T/2-1):    w_{t+1} = E^T (w_t  * s_t),  w_0 = exp(trans[START,:])
  backward half (t = T-1..T/2):    r_t     = s_t * (E r_{t+1}), r_T = e_END  (adjoint)
  e_END^T w_T = r_{T/2}^T w_{T/2}  -> one dot product at the junction.
The two 256-step chains are independent, so each one's (DVE mul -> PE matmul ->
sem) latency hides inside the other's gaps: ~256 dependent rounds, not 512.
s_t = exp(scores_t - delta) is produced by the scalar engine (Exp only — no
activation-table thrashing).

Magnitude control: at each chunk boundary (CHS schedule) each chain measures
z = 1^T y via a tiny matmul; 1/z (vector reciprocal, bf16) is broadcast across
partitions by a K=1 matmul and applied as a one-step multiplicative jolt to
that chain's exp'd-score stream two chunks later. The f32 z values stream to
DRAM and the host adds back sum(log z) over the applied corrections:
  fs_T[b] = log(dot[b]) + T*delta + sum_c log zA[c,b] + sum_c log zB[c,b].

Per-core layout: tags on partitions (48), local batch (8) on the free dim.
mask is all ones per the problem spec (fill: ones), so the mask gating
(where(mask, nxt, fs)) is the identity and is not materialized on device.
"""

import numpy as np

B, T, L = 64, 512, 48
START, PAD, END = 46, 45, 47
NCORES = 8
BL = B // NCORES          # 8 batch elements per core
H = T // 2                # steps per chain
CHS = [8, 24] + [32] * 7  # renorm chunk sizes (small first chunk -> the first
                          # DMA+exp gating the chain start is tiny)
SOFF = [sum(CHS[:i]) for i in range(len(CHS))]
NCH2 = len(CHS)           # chunks per chain
LAG = 2                   # feedback delay (chunks) for the 1/z correction
DELTA = 5.0               # static per-step log shift folded into exp(scores)

_NC_CACHE = {}


def build_nc():
    import concourse.bacc as bacc
    import concourse.mybir as mybir
    import concourse.tile as tile

    f32 = mybir.dt.float32
    bf16 = mybir.dt.bfloat16
    AF = mybir.ActivationFunctionType
    AL = mybir.AluOpType
    AX = mybir.AxisListType

    nc = bacc.Bacc("TRN2", target_bir_lowering=False, debug=False)

    s_dram = nc.dram_tensor("s_tr", [L, T * BL], f32, kind="ExternalInput")
    trans_d = nc.dram_tensor("trans", [L, L], f32, kind="ExternalInput")
    transT_d = nc.dram_tensor("transT", [L, L], f32, kind="ExternalInput")
    goldf_d = nc.dram_tensor("goldf", [128, 32], f32, kind="ExternalInput")
    sc0_d = nc.dram_tensor("sc0", [128, 32], f32, kind="ExternalInput")
    iota_d = nc.dram_tensor("iotaf", [128, L], f32, kind="ExternalInput")

    # one output tensor: [ zA(128) | zB(128) | dot(8) | tg_gather, sc0_sum ]
    out_d = nc.dram_tensor(
        "out_all", [1, 2 * NCH2 * BL + BL + 2], f32, kind="ExternalOutput")

    with tile.TileContext(nc) as tc:
        with (
            tc.tile_pool(name="const", bufs=1) as cpool,
            tc.tile_pool(name="sraw", bufs=4) as rpool,
            tc.tile_pool(name="sexp", bufs=4) as epool,
            tc.tile_pool(name="yy", bufs=4) as ypool,
            tc.tile_pool(name="small", bufs=4) as smpool,
            tc.tile_pool(name="oh", bufs=2) as ohpool,
            tc.tile_pool(name="wps", bufs=2, space="PSUM") as wpool,
            tc.tile_pool(name="qps", bufs=2, space="PSUM") as qpool,
            tc.tile_pool(name="zps", bufs=2, space="PSUM") as zpool,
            tc.tile_pool(name="cntps", bufs=1, space="PSUM") as cntpool,
            tc.tile_pool(name="cbps", bufs=1, space="PSUM") as cbpool,
        ):
            # ---- startup-critical DMAs first (transT gates the chain-A init
            # scalar; raw chunks gate the first steps); other constants ride
            # the (otherwise idle) gpsimd DMA queue ----
            raw0A = rpool.tile([L, CHS[0] * BL], f32, tag="rawA")
            nc.sync.dma_start(raw0A[:], s_dram[:, 0:CHS[0] * BL])
            transT_sb = cpool.tile([L, L], f32)
            nc.sync.dma_start(transT_sb[:], transT_d[:])

            trans_sb = cpool.tile([L, L], f32)
            nc.gpsimd.dma_start(trans_sb[:], trans_d[:])
            raw0B = rpool.tile([L, CHS[0] * BL], f32, tag="rawB")
            nc.gpsimd.dma_start(raw0B[:], s_dram[:, (T - CHS[0]) * BL:T * BL])

            zero48 = cpool.tile([L, 1], f32)
            nc.vector.memset(zero48[:], 0.0)
            negd48 = cpool.tile([L, 1], f32)
            nc.vector.memset(negd48[:], -DELTA)

            # prefetch the Exp activation table while the DMAs run
            warm_act = cpool.tile([L, 1], f32)
            nc.scalar.activation(warm_act[:], zero48[:], AF.Exp, bias=zero48[:])

            # only two f32 columns are needed for the chain inits:
            # exp(trans[START,:]) = exp(transT[:,START]) and exp(trans[:,END]);
            # (48,1) exps keep the startup ACT train short
            ETcol = cpool.tile([L, 1], f32)
            nc.scalar.activation(
                ETcol[:], transT_sb[:, START:START + 1], AF.Exp, bias=zero48[:])
            E_bf = cpool.tile([L, L], bf16)
            nc.scalar.activation(E_bf[:], trans_sb[:], AF.Exp, bias=zero48[:])
            Ecol = cpool.tile([L, 1], f32)
            nc.scalar.activation(
                Ecol[:], trans_sb[:, END:END + 1], AF.Exp, bias=zero48[:])
            ET_bf = cpool.tile([L, L], bf16)
            nc.scalar.activation(ET_bf[:], transT_sb[:], AF.Exp, bias=zero48[:])

            ones48b = cpool.tile([L, 1], bf16)
            nc.vector.memset(ones48b[:], 1.0)
            ones48f = cpool.tile([L, 1], f32)
            nc.vector.memset(ones48f[:], 1.0)
            ones1x48 = cpool.tile([1, L], bf16)
            nc.vector.memset(ones1x48[:], 1.0)
            ones128b = cpool.tile([128, 1], bf16)
            nc.vector.memset(ones128b[:], 1.0)
            ones128f = cpool.tile([128, 1], f32)
            nc.vector.memset(ones128f[:], 1.0)
            zbuf = cpool.tile([1, 2 * NCH2 * BL + BL + 2], f32)
            ZD = 2 * NCH2 * BL          # dot offset in zbuf
            ZS = ZD + BL                # (tg_gather, sc0_sum) offset

            # gold-histogram inputs (consumed from chunk 0 onward, off-path)
            iota_sb = cpool.tile([128, L], f32)
            nc.gpsimd.dma_start(iota_sb[:], iota_d[:])
            goldf_sb = cpool.tile([128, 32], f32)
            nc.gpsimd.dma_start(goldf_sb[:], goldf_d[:])
            sc0_sb = cpool.tile([128, 32], f32)
            nc.gpsimd.dma_start(sc0_sb[:], sc0_d[:])
            cnt_ps = cntpool.tile([L, 1], f32)

            # ---- twin 256-step chains, interleaved ----
            cbA, cbB = {}, {}
            w_prev = None     # chain A state (PSUM)
            q_prev = None     # chain B state (PSUM)
            yA = yB = None
            pending_fb = []   # feedback ops deferred into the next chunk so
                              # the in-order PE queue isn't head-blocked on DVE
            pending_z = []    # z-measure matmuls, deferred one round likewise
            hist_cc = 0
            for c in range(NCH2):
                K = CHS[c]
                s0, s1 = SOFF[c], SOFF[c] + K
                if c == 0:
                    rawA, rawB = raw0A, raw0B
                else:
                    rawA = rpool.tile([L, K * BL], f32, tag="rawA")
                    nc.sync.dma_start(rawA[:], s_dram[:, s0 * BL:s1 * BL])
                    rawB = rpool.tile([L, K * BL], f32, tag="rawB")
                    nc.sync.dma_start(
                        rawB[:], s_dram[:, (T - s1) * BL:(T - s0) * BL])
                seA = epool.tile([L, K, BL], f32, tag="seA")
                nc.scalar.activation(
                    seA[:].rearrange("p a b -> p (a b)"), rawA[:], AF.Exp,
                    bias=negd48[:])
                seB = epool.tile([L, K, BL], f32, tag="seB")
                nc.scalar.activation(
                    seB[:].rearrange("p a b -> p (a b)"), rawB[:], AF.Exp,
                    bias=negd48[:])

                if c in cbA:
                    s0cA = smpool.tile([L, BL], f32, tag="s0cA")
                    nc.vector.tensor_tensor(
                        s0cA[:], seA[:, 0, :], cbA.pop(c)[:, 0:BL], AL.mult)
                else:
                    s0cA = None
                if c in cbB:
                    s0cB = smpool.tile([L, BL], f32, tag="s0cB")
                    nc.vector.tensor_tensor(
                        s0cB[:], seB[:, K - 1, :], cbB.pop(c)[:, BL:2 * BL],
                        AL.mult)
                else:
                    s0cB = None

                for k in range(K):
                    if k == 1 and pending_z:
                        for zb in pending_z:
                            zb()
                        pending_z = []
                    if k == 6 and pending_fb:
                        for fb in pending_fb:
                            fb()
                        pending_fb = []
                    if c >= 1 and k % 6 == 3 and hist_cc < 32:
                        # fold gold-histogram pieces into the chunks'
                        # DVE/PE slack instead of a serial tail
                        cc = hist_cc
                        hist_cc += 1
                        oh = ohpool.tile([128, L], bf16, tag="oh")
                        nc.vector.tensor_scalar(
                            oh[:], iota_sb[:], goldf_sb[:, cc:cc + 1], None,
                            AL.is_equal)
                        nc.tensor.matmul(
                            cnt_ps[:], oh[:], ones128b[:],
                            start=(cc == 0), stop=(cc == 31))
                        if cc == 31:
                            # tg epilogue, inside the loop so it overlaps the
                            # final chunk instead of serializing after it
                            cnt_sb = smpool.tile([L, 1], f32, tag="cnt")
                            nc.vector.tensor_copy(cnt_sb[:], cnt_ps[:])
                            tgg_ps = zpool.tile([1, 1], f32, tag="z")
                            nc.tensor.matmul(
                                tgg_ps[:], cnt_sb[:], transT_sb[:, 0:1],
                                start=True, stop=True)
                            nc.vector.tensor_copy(
                                zbuf[:, ZS:ZS + 1], tgg_ps[:])
                            red = smpool.tile([128, 1], f32, tag="red")
                            nc.vector.reduce_sum(red[:], sc0_sb[:], axis=AX.X)
                            sc_ps = zpool.tile([1, 1], f32, tag="z")
                            nc.tensor.matmul(
                                sc_ps[:], red[:], ones128f[:],
                                start=True, stop=True)
                            nc.vector.tensor_copy(
                                zbuf[:, ZS + 1:ZS + 2], sc_ps[:])
                    # chain A, step = s0 + k (ascending t)
                    sA = s0cA[:] if (k == 0 and s0cA is not None) else seA[:, k, :]
                    yA = ypool.tile([L, BL], bf16, tag="yA")
                    if c == 0 and k == 0:
                        nc.vector.tensor_scalar_mul(
                            yA[:], sA, ETcol[:])
                    else:
                        nc.vector.tensor_tensor(yA[:], w_prev[:], sA, AL.mult)
                    w_prev = wpool.tile([L, BL], f32, tag="w")
                    nc.tensor.matmul(
                        w_prev[:], E_bf[:], yA[:], start=True, stop=True)

                    # chain B, t = T-1 - (c*K + k) (descending); kk indexes seB
                    kk = K - 1 - k
                    last_b = (c == NCH2 - 1 and k == K - 1)
                    sB = s0cB[:] if (k == 0 and s0cB is not None) else seB[:, kk, :]
                    yB = ypool.tile([L, BL], f32 if last_b else bf16, tag="yB")
                    if c == 0 and k == 0:
                        nc.vector.tensor_scalar_mul(
                            yB[:], sB, Ecol[:])
                    else:
                        nc.vector.tensor_tensor(yB[:], q_prev[:], sB, AL.mult)
                    if not last_b:      # r_{T/2} itself never enters a matmul
                        q_prev = qpool.tile([L, BL], f32, tag="q")
                        nc.tensor.matmul(
                            q_prev[:], ET_bf[:], yB[:], start=True, stop=True)

                # chunk-end magnitude measurement + delayed 1/z feedback;
                # both chains' broadcast factors share one PSUM tile (A|B).
                # The z matmuls + copies run one round into the next chunk and
                # the reciprocal + broadcast five rounds later, so neither the
                # PE nor the DVE queue head-blocks at the chunk seam.
                if c + LAG < NCH2:
                    cbt = cbpool.tile([L, 2 * BL], f32, tag="cb")
                else:
                    cbt = None
                last_chunk = (c == NCH2 - 1)
                for name, ytile, cbmap, zoff, cbsl in (
                        ("A", yA, cbA, c * BL, (0, BL)),
                        ("B", yB, cbB, (NCH2 + c) * BL, (BL, 2 * BL))):
                    def _zb(name=name, ytile=ytile, zoff=zoff, cbsl=cbsl,
                            cbt=cbt, cbmap=cbmap, c=c, last_chunk=last_chunk):
                        z_ps = zpool.tile([1, BL], f32, tag="z")
                        lhs1 = ones48f if (name == "B" and last_chunk) else ones48b
                        nc.tensor.matmul(
                            z_ps[:], lhs1[:], ytile[:], start=True, stop=True)
                        nc.vector.tensor_copy(zbuf[:, zoff:zoff + BL], z_ps[:])
                        if cbt is not None:
                            def _fb(z_ps=z_ps, cbt=cbt, cbsl=cbsl, name=name):
                                zr = smpool.tile([1, BL], bf16, tag="zr" + name)
                                # bf16 rounding of the 1/z factor only shifts
                                # which factor was applied; harmless (log z is
                                # re-added on the host from the f32 z_out)
                                with nc.allow_low_precision(
                                        reason="renorm factor"):
                                    nc.vector.reciprocal(zr[:], z_ps[:])
                                nc.tensor.matmul(
                                    cbt[:, cbsl[0]:cbsl[1]], ones1x48[:], zr[:],
                                    start=True, stop=True)
                            pending_fb.append(_fb)
                            cbmap[c + LAG] = cbt
                    if last_chunk:
                        _zb()
                    else:
                        pending_z.append(_zb)

            # junction dot product: e_END^T w_T = r_{T/2}^T w_{T/2}
            dprod = smpool.tile([L, BL], f32, tag="dprod")
            nc.vector.tensor_tensor(dprod[:], w_prev[:], yB[:], AL.mult)
            d_ps = zpool.tile([1, BL], f32, tag="z")
            nc.tensor.matmul(d_ps[:], ones48f[:], dprod[:], start=True, stop=True)
            nc.vector.tensor_copy(zbuf[:, ZD:ZD + BL], d_ps[:])
            nc.sync.dma_start(out_d[:], zbuf[:])

    nc.compile()
    return nc


def _get_nc():
    if "nc" not in _NC_CACHE:
        _NC_CACHE["nc"] = build_nc()
    return _NC_CACHE["nc"]


def make_in_maps(scores, gold_target, transitions):
    scores = np.asarray(scores, dtype=np.float32)
    gold = np.asarray(gold_target)
    trans = np.ascontiguousarray(np.asarray(transitions, dtype=np.float32))
    transT = np.ascontiguousarray(trans.T)
    iota = np.ascontiguousarray(
        np.broadcast_to(np.arange(L, dtype=np.float32)[None, :], (128, L)))
    in_maps = []
    for c in range(NCORES):
        sc = scores[c * BL:(c + 1) * BL]                     # (BL, T, L)
        s_tr = np.ascontiguousarray(sc.transpose(2, 1, 0)).reshape(L, T * BL)
        goldf = np.ascontiguousarray(
            gold[c * BL:(c + 1) * BL].astype(np.float32).reshape(128, 32))
        sc0 = np.ascontiguousarray(sc[:, :, 0].astype(np.float32).reshape(128, 32))
        in_maps.append({
            "s_tr": s_tr, "trans": trans, "transT": transT,
            "goldf": goldf, "sc0": sc0, "iotaf": iota,
        })
    return in_maps


def combine_outputs(results, transitions):
    trans = np.asarray(transitions, dtype=np.float64)
    forscore = 0.0
    tg_energy = 0.0
    nz = 2 * NCH2 * BL
    for c in range(NCORES):
        out = np.asarray(results[c]["out_all"], dtype=np.float64)[0]
        zv = out[:nz].reshape(2, NCH2, BL)
        dv = out[nz:nz + BL]
        tgg, sc0s = out[nz + BL], out[nz + BL + 1]
        fs_end = (np.log(dv) + DELTA * T
                  + np.log(zv[0, :NCH2 - LAG]).sum(axis=0)
                  + np.log(zv[1, :NCH2 - LAG]).sum(axis=0))
        forscore += fs_end.sum()
        tg_energy += tgg + sc0s + BL * trans[0, START]
    return np.float32((forscore - tg_energy) / B)


def kernel(scores, gold_target, mask, transitions):
    from concourse.bass_utils import run_bass_kernel_spmd

    nc = _get_nc()
    in_maps = make_in_maps(scores, gold_target, transitions)
    res = run_bass_kernel_spmd(nc, in_maps, list(range(NCORES)))
    return combine_outputs(res.results, transitions)



# revision 7
# speedup vs baseline: 1.0000x; 1.0000x over previous
"""CRF loss kernel for Trainium2 (8 NeuronCores, data-parallel over batch).

Math (faithful to the reference):
  loss = (forscore - tg_energy) / B
  tg_energy = B*trans[0,START] + sum_bt scores[b,t,0] + sum_bt trans[0, gold[b,t]]
    (the reference's torch.gather-on-flattened-(L*L) quirk reduces to row 0;
     computed on the host -- it is pure input-side math)
  forscore = sum_b fs_T[b, END], where fs is the standard CRF forward recurrence
    fs_{t+1}[j] = logsumexp_i(fs_t[i] + scores[t,i] + trans[i,j]), fs_0 = trans[START,:]

Device algorithm, linear space with E = exp(trans) (bf16 matmuls, f32 PSUM):
  forward half  (t = 0..T/2-1):    w_{t+1} = E^T (w_t  * s_t),  w_0 = exp(trans[START,:])
  backward half (t = T-1..T/2):    r_t     = s_t * (E r_{t+1}), r_T = e_END  (adjoint)
  e_END^T w_T = y_{T/2}^T w_{T/2}  -> dot product at the junction (on host, f64).

The two chains are STACKED on 96 partitions: state X = [w; r], one elementwise
multiply (state * exp-scores) and one matmul with block-diagonal weights
W = [[E, 0], [0, E^T]] per step.  The host pre-reverses the backward half of
the score stream so both halves consume ascending columns.  The 8-element
local batch is split 4+4 into two independent stacked pairs so each pair's
(mult -> matmul) latency hides inside the other's slack; pair 0's multiplies
run on the Vector engine, pair 1's on the (otherwise idle) GpSimd/Pool engine.
s_t = exp(scores_t - DELTA) is produced by the scalar engine.

Magnitude control: at each chunk boundary (CHS schedule) a tiny matmul
SEL2^T y measures zA = sum(y[0:48]), zB = sum(y[48:96]) per batch column for
both pairs at once; 1/z (vector reciprocal, bf16) is broadcast across
partitions by a K=2 matmul and applied as a one-step multiplicative jolt to
the exp'd-score stream two chunks later.  The f32 z values are staged to SBUF
(scalar-engine copies) and DMA'd out once at the end; the host adds back
sum(log z) over the applied corrections:
  fs_T[b] = log(dot[b]) + T*DELTA + sum_c log zA[c,b] + sum_c log zB[c,b].

Per-core layout: tags on partitions (2 x 48), local batch (8 = 4+4) on the
free dim.  mask is all ones per the problem spec (fill: ones), so the mask
gating (where(mask, nxt, fs)) is the identity and is not materialized.
"""

import numpy as np

B, T, L = 64, 512, 48
START, PAD, END = 46, 45, 47
NCORES = 8
BL = B // NCORES          # 8 batch elements per core
PB = BL // 2              # 4 batch elements per stacked pair
P2 = 2 * L                # 96 partitions: [forward chain; backward chain]
H = T // 2                # 256 steps per stacked pair
CHS = [8, 24] + [32] * 7  # renorm chunk sizes (small first chunk -> the first
                          # DMA+exp gating the chain start is tiny)
SOFF = [sum(CHS[:i]) for i in range(len(CHS))]
NCH = len(CHS)            # chunks
LAG = 2                   # feedback delay (chunks) for the 1/z correction
NZB = NCH - LAG           # boundaries with feedback/z-output (0..6)
DELTA = 5.0               # static per-step log shift folded into exp(scores)
OUTC = 8 * NZB + 2 * BL   # stage cols: z blocks + final (w|y) per pair

_NC_CACHE = {}


def build_nc():
    import concourse.bacc as bacc
    import concourse.mybir as mybir
    import concourse.tile as tile

    f32 = mybir.dt.float32
    bf16 = mybir.dt.bfloat16
    AF = mybir.ActivationFunctionType
    AL = mybir.AluOpType

    nc = bacc.Bacc("TRN2", target_bir_lowering=False, debug=False)

    s_dram = nc.dram_tensor("s_stack", [P2, H * BL], f32, kind="ExternalInput")
    wf_dram = nc.dram_tensor("wf", [P2, P2 + 3], f32, kind="ExternalInput")
    sb_dram = nc.dram_tensor("selbf", [2, P2], f32, kind="ExternalInput")
    out_d = nc.dram_tensor("stage_out", [P2, OUTC], f32, kind="ExternalOutput")

    with tile.TileContext(nc) as tc:
        with (
            tc.tile_pool(name="const", bufs=1) as cpool,
            tc.tile_pool(name="sraw", bufs=3) as rpool,
            tc.tile_pool(name="sexp", bufs=4) as epool,
            tc.tile_pool(name="y0", bufs=6) as y0pool,
            tc.tile_pool(name="y1", bufs=6) as y1pool,
            tc.tile_pool(name="small", bufs=4) as smpool,
            tc.tile_pool(name="st0", bufs=3, space="PSUM") as s0pool,
            tc.tile_pool(name="st1", bufs=3, space="PSUM") as s1pool,
            tc.tile_pool(name="zps", bufs=1, space="PSUM") as zpool,
            tc.tile_pool(name="cbps", bufs=1, space="PSUM") as cbpool,
        ):
            # ---- startup: chunk-0 scores DMA on the fast queue; constants on
            # the gpsimd queue; Exp table load warms in parallel ----
            raw0 = rpool.tile([P2, CHS[0] * BL], f32, tag="raw")
            nc.sync.dma_start(raw0[:], s_dram[:, 0:CHS[0] * BL])
            wf_sb = cpool.tile([P2, P2 + 3], f32)
            nc.gpsimd.dma_start(wf_sb[:], wf_dram[:])
            selb_f = cpool.tile([2, P2], f32)
            nc.gpsimd.dma_start(selb_f[:], sb_dram[:])

            zero96 = cpool.tile([P2, 1], f32)
            nc.vector.memset(zero96[:], 0.0)
            negd96 = cpool.tile([P2, 1], f32)
            nc.vector.memset(negd96[:], -DELTA)

            # prefetch the Exp activation table while the DMAs run
            warm = cpool.tile([P2, 1], f32)
            nc.scalar.activation(warm[:], zero96[:], AF.Exp, bias=zero96[:])

            # selection constants + weights to bf16 (Copy shares the
            # Exp table set); initc stays f32 inside the wf upload
            sel2 = cpool.tile([P2, 2], bf16)   # z measurement: [1_A | 1_B] cols
            nc.scalar.activation(sel2[:], wf_sb[:, P2:P2 + 2], AF.Copy, bias=0.0)
            selb = cpool.tile([2, P2], bf16)   # 1/z broadcast: rows -> halves
            nc.scalar.activation(selb[:], selb_f[:], AF.Copy, bias=0.0)
            initc = wf_sb[:, P2 + 2:P2 + 3]
            w_bf = cpool.tile([P2, P2], bf16)
            nc.scalar.activation(w_bf[:], wf_sb[:, 0:P2], AF.Copy, bias=0.0)

            stage = cpool.tile([P2, OUTC], f32)

            # exp'd scores for chunk 0
            raws = {0: raw0}
            ses = {}
            ses[0] = epool.tile([P2, CHS[0], BL], f32, tag="se", name="se0")
            nc.scalar.activation(
                ses[0][:].rearrange("p a b -> p (a b)"), raw0[:], AF.Exp,
                bias=negd96[:])

            st0 = st1 = None          # per-pair PSUM states
            y0 = y1 = None            # per-pair multiply outputs (SBUF bf16)
            s0c = {}                  # pair -> {chunk: corrected first column}
            s0c[0], s0c[1] = {}, {}
            ybnd = {}                 # boundary step y tiles (for z measure)
            z_ps = None
            zr = None
            cb = None

            for k in range(H):
                c = next(i for i in range(NCH)
                         if SOFF[i] <= k < SOFF[i] + CHS[i])
                kk = k - SOFF[c]
                K = CHS[c]
                bprev = c - 1             # boundary index finishing at c start

                # -- injected: prefetch next chunk (DMA early, exp late) --
                if kk == 2 and c + 1 < NCH:
                    Kn = CHS[c + 1]
                    raws[c + 1] = rpool.tile(
                        [P2, Kn * BL], f32, tag="raw", name=f"raw{c + 1}")
                    nc.sync.dma_start(
                        raws[c + 1][:],
                        s_dram[:, SOFF[c + 1] * BL:(SOFF[c + 1] + Kn) * BL])
                if kk == max(3, K - 10) and c + 1 < NCH:
                    Kn = CHS[c + 1]
                    ses[c + 1] = epool.tile(
                        [P2, Kn, BL], f32, tag="se", name=f"se{c + 1}")
                    nc.scalar.activation(
                        ses[c + 1][:].rearrange("p a b -> p (a b)"),
                        raws[c + 1][:], AF.Exp, bias=negd96[:])
                    raws.pop(c + 1 - 1, None)

                # -- injected: boundary bprev ops (z measure + 1/z feedback) --
                if 0 <= bprev < NZB:
                    if kk == 1:
                        yb0, yb1 = ybnd.pop(bprev)
                        z_ps = zpool.tile([2, 2 * PB], f32, tag="z")
                        nc.tensor.matmul(
                            z_ps[:, 0:PB], sel2[:], yb0[:],
                            start=True, stop=True, skip_group_check=True)
                        nc.tensor.matmul(
                            z_ps[:, PB:2 * PB], sel2[:], yb1[:],
                            start=True, stop=True, skip_group_check=True)
                    if kk == 3:
                        zr = smpool.tile([2, 2 * PB], bf16, tag="zr")
                        # bf16 rounding of the 1/z factor only shifts which
                        # factor was applied; log z is re-added on the host
                        # from the f32 staged copy
                        with nc.allow_low_precision(reason="renorm factor"):
                            nc.vector.reciprocal(zr[:], z_ps[:])
                        nc.scalar.activation(
                            stage[0:2, 8 * bprev:8 * bprev + 8], z_ps[:],
                            AF.Copy, bias=0.0)
                    if kk == 5:
                        cb = cbpool.tile([P2, 2 * PB], f32, tag="cb")
                        nc.tensor.matmul(
                            cb[:], selb[:], zr[:], start=True, stop=True)
                    if kk == 7:
                        cb_sb = smpool.tile([P2, 2 * PB], f32, tag="cbsb")
                        nc.scalar.activation(
                            cb_sb[:], cb[:], AF.Copy, bias=0.0)
                    if kk == K - 3:
                        # corrected first column for chunk c+1 (= bprev+LAG);
                        # SBUF-only, so it runs on the idle Pool engine
                        tgt = c + 1
                        t01 = smpool.tile([P2, 2 * PB], f32, tag="s0c")
                        nc.gpsimd.tensor_tensor(
                            t01[:], ses[tgt][:, 0, :], cb_sb[:], AL.mult)
                        s0c[0][tgt] = t01
                        s0c[1][tgt] = t01

                # -- the two stacked chain-pairs: mult (DVE / Pool), matmul --
                se = ses[c]
                in0_0 = (s0c[0].pop(c)[:, 0:PB] if (kk == 0 and c in s0c[0])
                         else se[:, kk, 0:PB])
                y0 = y0pool.tile([P2, PB], bf16, tag="y0")
                if k == 0:
                    nc.vector.tensor_scalar_mul(y0[:], in0_0, initc)
                else:
                    nc.vector.tensor_tensor(y0[:], st0[:], in0_0, AL.mult)

                in0_1 = (s0c[1].pop(c)[:, PB:2 * PB] if (kk == 0 and c in s0c[1])
                         else se[:, kk, PB:2 * PB])
                y1 = y1pool.tile([P2, PB], bf16, tag="y1")
                if k == 0:
                    nc.vector.tensor_scalar_mul(y1[:], in0_1, initc)
                else:
                    nc.vector.tensor_tensor(y1[:], st1[:], in0_1, AL.mult)

                st0 = s0pool.tile([P2, PB], f32, tag="st0")
                nc.tensor.matmul(st0[:], w_bf[:], y0[:], start=True, stop=True)
                st1 = s1pool.tile([P2, PB], f32, tag="st1")
                nc.tensor.matmul(st1[:], w_bf[:], y1[:], start=True, stop=True)

                if kk == K - 1 and c < NZB:
                    ybnd[c] = (y0, y1)

            # ---- tail: junction operands to stage, one DMA out ----
            # full-tile copies (base-0 partition access only); the host picks
            # w_T from rows 0:L of the state and y_{T/2} from rows L: of y
            cw = 8 * NZB
            nc.scalar.activation(
                stage[:, cw:cw + PB], st0[:], AF.Copy, bias=0.0)
            nc.scalar.activation(
                stage[:, cw + PB:cw + 2 * PB], st1[:], AF.Copy, bias=0.0)
            nc.scalar.activation(
                stage[:, cw + 2 * PB:cw + 3 * PB], y0[:], AF.Copy, bias=0.0)
            nc.scalar.activation(
                stage[:, cw + 3 * PB:cw + 4 * PB], y1[:], AF.Copy, bias=0.0)
            nc.sync.dma_start(out_d[:], stage[:])

    nc.compile()
    return nc


def _get_nc():
    if "nc" not in _NC_CACHE:
        _NC_CACHE["nc"] = build_nc()
    return _NC_CACHE["nc"]


def make_in_maps(scores, transitions):
    scores = np.asarray(scores, dtype=np.float32)
    trans = np.asarray(transitions, dtype=np.float64)
    E = np.exp(trans).astype(np.float32)
    wf = np.zeros((P2, P2 + 3), dtype=np.float32)
    wf[0:L, 0:L] = E
    wf[L:P2, L:L + L] = E.T
    wf[0:L, P2] = 1.0           # sel2 col 0: sum over the forward half
    wf[L:P2, P2 + 1] = 1.0      # sel2 col 1: sum over the backward half
    wf[0:L, P2 + 2] = np.exp(trans[START, :])     # initc
    wf[L:P2, P2 + 2] = np.exp(trans[:, END])
    wf = np.ascontiguousarray(wf)
    selbf = np.zeros((2, P2), dtype=np.float32)
    selbf[0, 0:L] = 1.0
    selbf[1, L:P2] = 1.0
    in_maps = []
    for cix in range(NCORES):
        sc = scores[cix * BL:(cix + 1) * BL]                 # (BL, T, L)
        fwd = sc[:, 0:H, :].transpose(2, 1, 0)               # (L, H, BL)
        bwd = sc[:, T - 1:H - 1:-1, :].transpose(2, 1, 0)    # t = T-1 .. H
        s_stack = np.ascontiguousarray(
            np.concatenate([fwd, bwd], axis=0).reshape(P2, H * BL))
        in_maps.append({"s_stack": s_stack, "wf": wf, "selbf": selbf})
    return in_maps


def combine_outputs(results, scores, gold_target, transitions):
    scores = np.asarray(scores, dtype=np.float64)
    gold = np.asarray(gold_target).reshape(-1)
    trans = np.asarray(transitions, dtype=np.float64)
    tg_energy = (B * trans[0, START] + scores[:, :, 0].sum()
                 + trans[0][gold].sum())
    forscore = 0.0
    cw = 8 * NZB
    for cix in range(NCORES):
        out = np.asarray(results[cix]["stage_out"], dtype=np.float64)
        zs = out[0:2, :cw].reshape(2, NZB, 2 * PB)   # [zA|zB, boundary, col]
        logz = np.log(zs).sum(axis=(0, 1))           # (8,) per pair-col
        w = out[0:L, cw:cw + 2 * PB]                 # (48, 8)
        y = out[L:P2, cw + 2 * PB:cw + 4 * PB]       # (48, 8)
        dot = (w * y).sum(axis=0)                    # (8,)
        forscore += (np.log(dot) + T * DELTA + logz).sum()
    return np.float32((forscore - tg_energy) / B)


def kernel(scores, gold_target, mask, transitions):
    from concourse.bass_utils import run_bass_kernel_spmd

    nc = _get_nc()
    in_maps = make_in_maps(scores, transitions)
    res = run_bass_kernel_spmd(nc, in_maps, list(range(NCORES)))
    return combine_outputs(res.results, scores, gold_target, transitions)


# revision 8
# speedup vs baseline: 2.0260x; 2.0260x over previous
"""CRF loss kernel for Trainium2 (8 NeuronCores, data-parallel over batch).

Math (faithful to the reference):
  loss = (forscore - tg_energy) / B
  tg_energy = B*trans[0,START] + sum_bt scores[b,t,0] + sum_bt trans[0, gold[b,t]]
    (the reference's torch.gather-on-flattened-(L*L) quirk reduces to row 0;
     computed on the host -- it is pure input-side math)
  forscore = sum_b fs_T[b, END], where fs is the standard CRF forward recurrence
    fs_{t+1}[j] = logsumexp_i(fs_t[i] + scores[t,i] + trans[i,j]), fs_0 = trans[START,:]

Device algorithm, linear space with E = exp(trans) (bf16 matmuls, f32 PSUM):
  w_{t+1} = E^T (w_t * s_t), s_t = exp(scores_t - DELTA) (host-computed, bf16).

The dependent chain w -> y -> w is latency-bound on TRN2 (~414 ns per step:
DVE multiply + PE matmul + semaphores), so the kernel SHORTENS THE CHAIN
instead of adding parallel batch work: products of positive matrices mix --
the state direction forgets its initial condition at ~e^-1.4/step here -- so
time is cut into S=6 segments and each mid-stream segment starts from an
all-ones state with a BURN=20-step burn-in (direction error ~e^-28, far below
bf16 noise).  The unknown per-segment scalars telescope through ratios of
1^T y measured at junctions (chain j's burn-in end coincides with chain
j-1's last step, so both measure the SAME functional of the same time point).
Chain 0 needs no burn-in: its exact init exp(trans[START,:]) is folded into
its first score column on the host.

Per core: 6 chains of LEN=102 steps, stacked 2-per-stream on 96 partitions
(rows 0:48 = chain 2m, rows 48:96 = chain 2m+1), with block-diagonal weights
W = [[E, 0], [0, E]].  3 streams interleave, so each stream's 414 ns step
latency hides inside the other two; per step: one DVE multiply [96,8] and one
PE matmul (the whole local batch of 8 rides in the free dim).

Magnitude control: renorm chunks [8,12,32,32,18]; at each chunk end a K=96
matmul with a two-ones-columns SEL matrix measures z = (1^T y) per chain; the
bf16 reciprocal is broadcast across partitions by a K=2 matmul and applied as
a one-step multiplicative jolt two chunks later (Pool-engine merge into the
score stream; Pool cannot touch PSUM, so the broadcast is staged to SBUF by
the scalar engine).  Boundary 1 sits exactly at the burn-in end, giving the
junction measurements for free.  f32 z values are staged and DMA'd out once;
the host assembles
  fs_b = sum_j [P_{j-1}(end) - P_j(burn)] + log w5[END] + applied-z logs + T*DELTA
with P_j(c) = log z_j,c + sum of chain j's corrections applied before c.

mask is all ones per the problem spec (fill: ones) and is not materialized.
"""

import numpy as np

B, T, L = 64, 512, 48
START, PAD, END = 46, 45, 47
NCORES = 8
BL = B // NCORES          # 8 batch elements per core (all ride the free dim)
P2 = 2 * L                # 96 partitions: two chains per stream
M = 3                     # streams (instruction-level independent chains)
S = 2 * M                 # time segments / chains
BURN = 20                 # burn-in steps for mid-stream chains
LEN = (T + (S - 1) * BURN) // S   # 102 steps per chain
SEG = LEN - BURN          # 82 real steps per mid-stream chain
CH = [8, BURN - 8, 32, 32, LEN - BURN - 64]   # renorm chunks: [8,12,32,32,18]
CUM = [sum(CH[:i + 1]) for i in range(len(CH))]  # [8,20,52,84,102]
CST = [0] + CUM[:-1]
NB = len(CH)              # 5 boundaries; feedback from boundaries 0..2
LAG = 2
DELTA = 5.0
OUTC = 3 * BL * NB + BL   # stage cols: z blocks (24 per boundary) + final state
A0 = [0] + [LEN + (j - 1) * SEG - BURN for j in range(1, S)]  # chain starts

_NC_CACHE = {}


def build_nc():
    import concourse.bacc as bacc
    import concourse.mybir as mybir
    import concourse.tile as tile

    f32 = mybir.dt.float32
    bf16 = mybir.dt.bfloat16
    AF = mybir.ActivationFunctionType
    AL = mybir.AluOpType

    nc = bacc.Bacc("TRN2", target_bir_lowering=False, debug=False)

    se_d = nc.dram_tensor("se_all", [P2, M * LEN * BL], bf16, kind="ExternalInput")
    ws_d = nc.dram_tensor("wsel", [P2, P2 + 2], bf16, kind="ExternalInput")
    sb_d = nc.dram_tensor("selbf", [2, P2], bf16, kind="ExternalInput")
    out_d = nc.dram_tensor("stage_out", [P2, OUTC], f32, kind="ExternalOutput")

    with tile.TileContext(nc) as tc:
        with (
            tc.tile_pool(name="const", bufs=1) as cpool,
            tc.tile_pool(name="sexp", bufs=2) as epool,
            tc.tile_pool(name="ys", bufs=6) as ypool,
            tc.tile_pool(name="small", bufs=3) as smpool,
            tc.tile_pool(name="st0", bufs=2, space="PSUM") as p0,
            tc.tile_pool(name="st1", bufs=2, space="PSUM") as p1,
            tc.tile_pool(name="st2", bufs=2, space="PSUM") as p2,
            tc.tile_pool(name="zps", bufs=1, space="PSUM") as zpool,
            tc.tile_pool(name="cbps", bufs=1, space="PSUM") as cbpool,
        ):
            spools = [p0, p1, p2]

            # ---- startup: first score chunks on the fast queue, constants on
            # the gpsimd queue; no activation tables needed (host pre-exps) ----
            ses = {}
            for m in range(M):
                t0 = epool.tile([P2, CH[0], BL], bf16, tag=f"se{m}",
                                name=f"se{m}_0")
                nc.sync.dma_start(
                    t0[:].rearrange("p a b -> p (a b)"),
                    se_d[:, m * LEN * BL:m * LEN * BL + CH[0] * BL])
                ses[(m, 0)] = t0
            ws_sb = cpool.tile([P2, P2 + 2], bf16)
            nc.gpsimd.dma_start(ws_sb[:], ws_d[:])
            selb = cpool.tile([2, P2], bf16)
            nc.gpsimd.dma_start(selb[:], sb_d[:])
            w_bf = ws_sb[:, 0:P2]
            sel2 = ws_sb[:, P2:P2 + 2]
            stage = cpool.tile([P2, OUTC], f32)

            st = [None] * M           # per-stream PSUM states
            ys = [None] * M           # per-stream multiply outputs
            ybnd = {}                 # (m, bc) -> boundary-step y tile
            s0c = {}                  # (m, chunk) -> corrected first column
            z_ps = {}                 # bc -> shared [2, 24] z PSUM tile
            zr = cb = cb_sb = None
            mrg_at = {}               # k -> (boundary, stream) merge schedule
            for bc in range(3):
                for m in range(M):
                    mrg_at[CST[bc + LAG] - 3 + m] = (bc, m)

            for k in range(LEN):
                c = next(i for i in range(NB) if CST[i] <= k < CUM[i])
                kk = k - CST[c]

                # -- prefetch next chunk (one DMA per stream, staggered) --
                if c + 1 < NB and kk in (2, 3, 4):
                    m = kk - 2
                    Kn = CH[c + 1]
                    tnext = epool.tile([P2, Kn, BL], bf16, tag=f"se{m}",
                                       name=f"se{m}_{c + 1}")
                    off = m * LEN * BL + CST[c + 1] * BL
                    nc.sync.dma_start(
                        tnext[:].rearrange("p a b -> p (a b)"),
                        se_d[:, off:off + Kn * BL])
                    ses[(m, c + 1)] = tnext

                # -- boundary bc event schedule (z at kb+2, feedback later) --
                for bc in range(NB - 1):
                    kb = CUM[bc] - 1
                    if k == kb + 2:
                        zt = zpool.tile([2, M * BL], f32, tag="z",
                                        name=f"z{bc}")
                        z_ps[bc] = zt
                        for m in range(M):
                            ym = ybnd.pop((m, bc))
                            nc.tensor.matmul(
                                zt[:, m * BL:(m + 1) * BL], sel2, ym[:],
                                start=True, stop=True, skip_group_check=True)
                    if k == kb + 3:
                        nc.scalar.activation(
                            stage[0:2, M * BL * bc:M * BL * (bc + 1)],
                            z_ps[bc][:], AF.Copy, bias=0.0)
                    if bc < 3 and k == kb + 4:
                        zr = smpool.tile([2, M * BL], bf16, tag="zr")
                        # bf16 rounding of the applied factor is absorbed by
                        # the telescoping; the host uses the staged f32 z
                        with nc.allow_low_precision(reason="renorm factor"):
                            nc.vector.reciprocal(zr[:], z_ps[bc][:])
                    if bc < 3 and k == kb + 5:
                        cb = cbpool.tile([P2, M * BL], f32, tag="cb")
                        nc.tensor.matmul(
                            cb[:], selb[:], zr[:], start=True, stop=True)
                    if bc < 3 and k == kb + 6:
                        cb_sb = smpool.tile([P2, M * BL], bf16, tag="cbsb")
                        nc.scalar.activation(cb_sb[:], cb[:], AF.Copy, bias=0.0)

                # -- feedback merges (Pool: SBUF-only) for chunk bc+LAG --
                if k in mrg_at:
                    bc, m = mrg_at[k]
                    tgt = bc + LAG
                    t01 = smpool.tile([P2, BL], bf16, tag=f"s0c{m}",
                                      name=f"s0c{m}_{tgt}")
                    nc.gpsimd.tensor_tensor(
                        t01[:], ses[(m, tgt)][:, 0, :],
                        cb_sb[:, m * BL:(m + 1) * BL], AL.mult)
                    s0c[(m, tgt)] = t01

                # -- the three stacked streams: multiply (DVE) + matmul --
                for m in range(M):
                    se = ses[(m, c)]
                    if k == 0:
                        rhs = se[:, 0, :]     # all-ones init: y_0 = s_0
                    else:
                        in1 = (s0c.pop((m, c))[:] if (kk == 0 and (m, c) in s0c)
                               else se[:, kk, :])
                        ym = ypool.tile([P2, BL], bf16, tag=f"y{m}",
                                        name=f"y{m}_{k}")
                        nc.vector.tensor_tensor(ym[:], st[m][:], in1, AL.mult)
                        ys[m] = ym
                        rhs = ym[:]
                        if k == CUM[c] - 1:
                            ybnd[(m, c)] = ym
                    st[m] = spools[m].tile([P2, BL], f32, tag=f"st{m}",
                                           name=f"st{m}_{k}")
                    nc.tensor.matmul(
                        st[m][:], w_bf, rhs, start=True, stop=True)

            # ---- tail: last boundary z, final state, one DMA out ----
            bc = NB - 1
            zt = zpool.tile([2, M * BL], f32, tag="z", name="z_last")
            for m in range(M):
                ym = ybnd.pop((m, bc))
                nc.tensor.matmul(
                    zt[:, m * BL:(m + 1) * BL], sel2, ym[:],
                    start=True, stop=True, skip_group_check=True)
            nc.scalar.activation(
                stage[0:2, M * BL * bc:M * BL * (bc + 1)], zt[:],
                AF.Copy, bias=0.0)
            nc.scalar.activation(
                stage[:, M * BL * NB:OUTC], st[M - 1][:], AF.Copy, bias=0.0)
            nc.sync.dma_start(out_d[:], stage[:])

    nc.compile()
    return nc


def _get_nc():
    if "nc" not in _NC_CACHE:
        _NC_CACHE["nc"] = build_nc()
    return _NC_CACHE["nc"]


def make_in_maps(scores, transitions):
    import ml_dtypes

    bf16 = ml_dtypes.bfloat16
    scores = np.asarray(scores, dtype=np.float64)
    trans = np.asarray(transitions, dtype=np.float64)
    E = np.exp(trans)
    wf = np.zeros((P2, P2 + 2), dtype=np.float64)
    wf[0:L, 0:L] = E
    wf[L:P2, L:P2] = E
    wf[0:L, P2] = 1.0            # sel2 col 0: 1^T over chain 2m
    wf[L:P2, P2 + 1] = 1.0       # sel2 col 1: 1^T over chain 2m+1
    wsel = np.ascontiguousarray(wf.astype(bf16))
    selbf = np.zeros((2, P2), dtype=np.float64)
    selbf[0, 0:L] = 1.0
    selbf[1, L:P2] = 1.0
    selbf = np.ascontiguousarray(selbf.astype(bf16))

    w0 = np.exp(trans[START, :])                 # chain-0 exact init
    in_maps = []
    for cix in range(NCORES):
        sc = scores[cix * BL:(cix + 1) * BL]     # (BL, T, L) f64
        se = np.empty((P2, M * LEN, BL), dtype=np.float64)
        for j in range(S):
            mm, r = j // 2, j % 2
            blk = np.exp(
                sc[:, A0[j]:A0[j] + LEN, :] - DELTA).transpose(2, 1, 0)
            if j == 0:
                blk = blk.copy()
                blk[:, 0, :] *= w0[:, None]
            se[r * L:(r + 1) * L, mm * LEN:(mm + 1) * LEN, :] = blk
        se = np.ascontiguousarray(
            se.reshape(P2, M * LEN * BL).astype(bf16))
        in_maps.append({"se_all": se, "wsel": wsel, "selbf": selbf})
    return in_maps


def combine_outputs(results, scores, gold_target, transitions):
    scores = np.asarray(scores, dtype=np.float64)
    gold = np.asarray(gold_target).reshape(-1)
    trans = np.asarray(transitions, dtype=np.float64)
    tg_energy = (B * trans[0, START] + scores[:, :, 0].sum()
                 + trans[0][gold].sum())
    forscore = 0.0
    for cix in range(NCORES):
        out = np.asarray(results[cix]["stage_out"], dtype=np.float64)
        # z[r, bc, m, b]: row r of stream m at boundary bc
        zs = out[0:2, :M * BL * NB].reshape(2, NB, M, BL)
        logz = np.log(zs)                        # (2, NB, M, BL)

        def lz(j, c):
            return logz[j % 2, c, j // 2]        # (BL,)

        def Pf(j, c):
            v = lz(j, c).copy()
            for cp in range(NB):
                if cp + LAG <= c:
                    v += lz(j, cp)
            return v

        la = np.zeros(BL)
        for j in range(1, S):
            la += Pf(j - 1, NB - 1) - Pf(j, 1)
        wfin = out[L + END, M * BL * NB:OUTC]    # chain S-1 END component (BL,)
        fs_b = la + np.log(wfin) + sum(lz(S - 1, cp) for cp in range(3)) \
            + T * DELTA
        forscore += fs_b.sum()
    return np.float32((forscore - tg_energy) / B)


def kernel(scores, gold_target, mask, transitions):
    from concourse.bass_utils import run_bass_kernel_spmd

    nc = _get_nc()
    in_maps = make_in_maps(scores, transitions)
    res = run_bass_kernel_spmd(nc, in_maps, list(range(NCORES)))
    return combine_outputs(res.results, scores, gold_target, transitions)


# revision 9
# speedup vs baseline: 2.1893x; 1.0806x over previous
"""CRF loss kernel for Trainium2 (8 NeuronCores, data-parallel over batch).

Math (faithful to the reference):
  loss = (forscore - tg_energy) / B
  tg_energy = B*trans[0,START] + sum_bt scores[b,t,0] + sum_bt trans[0, gold[b,t]]
    (the reference's torch.gather-on-flattened-(L*L) quirk reduces to row 0;
     computed on the host -- it is pure input-side math)
  forscore = sum_b fs_T[b, END], where fs is the standard CRF forward recurrence
    fs_{t+1}[j] = logsumexp_i(fs_t[i] + scores[t,i] + trans[i,j]), fs_0 = trans[START,:]

Device algorithm, linear space with E = exp(trans) (bf16 matmuls, f32 PSUM):
  w_{t+1} = E^T (w_t * s_t), s_t = exp(scores_t - DELTA) (host-computed, bf16).

The dependent chain w -> y -> w is latency-bound on TRN2 (~414 ns per step:
DVE multiply + PE matmul + semaphores), so the kernel SHORTENS THE CHAIN
instead of adding parallel batch work: products of positive matrices mix --
the state direction forgets its initial condition at ~e^-1.4/step here -- so
time is cut into S=6 segments and each mid-stream segment starts from an
all-ones state with a BURN=8-step burn-in (direction error ~e^-11, far below
bf16 noise).  The unknown per-segment scalars telescope through ratios of
1^T y measured at junctions (chain j's burn-in end coincides with chain
j-1's last step, so both measure the SAME functional of the same time point).
Chain 0 needs no burn-in: its exact init exp(trans[START,:]) is folded into
its first score column on the host.

Per core: 6 chains of LEN=92 steps, stacked 2-per-stream on 96 partitions
(rows 0:48 = chain 2m, rows 48:96 = chain 2m+1), with block-diagonal weights
W = [[E, 0], [0, E]].  3 streams interleave, so each stream's 414 ns step
latency hides inside the other two; per step: one DVE multiply [96,8] and one
PE matmul (the whole local batch of 8 rides in the free dim).

Magnitude control: renorm chunks [8,32,32,20]; at each chunk end a K=96
matmul with a two-ones-columns SEL matrix measures z = (1^T y) per chain; the
bf16 reciprocal is broadcast across partitions by a K=2 matmul and applied as
a one-step multiplicative jolt two chunks later (Pool-engine merge into the
score stream; Pool cannot touch PSUM, so the broadcast is staged to SBUF by
the scalar engine).  Boundary 1 sits exactly at the burn-in end, giving the
junction measurements for free.  f32 z values are staged and DMA'd out once;
the host assembles
  fs_b = sum_j [P_{j-1}(end) - P_j(burn)] + log w5[END] + applied-z logs + T*DELTA
with P_j(c) = log z_j,c + sum of chain j's corrections applied before c.

mask is all ones per the problem spec (fill: ones) and is not materialized.
"""

import numpy as np

B, T, L = 64, 512, 48
START, PAD, END = 46, 45, 47
NCORES = 8
BL = B // NCORES          # 8 batch elements per core (all ride the free dim)
P2 = 2 * L                # 96 partitions: two chains per stream
M = 3                     # streams (instruction-level independent chains)
S = 2 * M                 # time segments / chains
BURN = 8                  # burn-in steps for mid-stream chains
LEN = (T + (S - 1) * BURN) // S   # 92 steps per chain
SEG = LEN - BURN          # 84 real steps per mid-stream chain
CH = [8, 32, 32, LEN - 72]        # renorm chunks: [8,32,32,20]
CUM = [sum(CH[:i + 1]) for i in range(len(CH))]  # [8,40,72,92]
CST = [0] + CUM[:-1]
NB = len(CH)              # 4 boundaries; feedback from boundaries 0..NFB-1
LAG = 2
NFB = NB - LAG            # boundaries that feed a correction back
JB = CUM.index(BURN)      # boundary index at the burn-in end (junction)
DELTA = 5.0
OUTC = 3 * BL * NB + BL   # stage cols: z blocks (24 per boundary) + final state
A0 = [0] + [LEN + (j - 1) * SEG - BURN for j in range(1, S)]  # chain starts

_NC_CACHE = {}


def build_nc():
    import concourse.bacc as bacc
    import concourse.mybir as mybir
    import concourse.tile as tile

    f32 = mybir.dt.float32
    bf16 = mybir.dt.bfloat16
    AF = mybir.ActivationFunctionType
    AL = mybir.AluOpType

    nc = bacc.Bacc("TRN2", target_bir_lowering=False, debug=False)

    se_d = nc.dram_tensor("se_all", [P2, M * LEN * BL], bf16, kind="ExternalInput")
    ws_d = nc.dram_tensor("wsel", [P2, P2 + 2], bf16, kind="ExternalInput")
    sb_d = nc.dram_tensor("selbf", [2, P2], bf16, kind="ExternalInput")
    out_d = nc.dram_tensor("stage_out", [P2, OUTC], f32, kind="ExternalOutput")

    with tile.TileContext(nc) as tc:
        with (
            tc.tile_pool(name="const", bufs=1) as cpool,
            tc.tile_pool(name="sexp", bufs=2) as epool,
            tc.tile_pool(name="ys", bufs=6) as ypool,
            tc.tile_pool(name="small", bufs=3) as smpool,
            tc.tile_pool(name="st0", bufs=2, space="PSUM") as p0,
            tc.tile_pool(name="st1", bufs=2, space="PSUM") as p1,
            tc.tile_pool(name="st2", bufs=2, space="PSUM") as p2,
            tc.tile_pool(name="zps", bufs=1, space="PSUM") as zpool,
            tc.tile_pool(name="cbps", bufs=1, space="PSUM") as cbpool,
        ):
            spools = [p0, p1, p2]

            # ---- startup: first score chunks on the fast queue, constants on
            # the gpsimd queue; no activation tables needed (host pre-exps) ----
            ses = {}
            for m in range(M):
                t0 = epool.tile([P2, CH[0], BL], bf16, tag=f"se{m}",
                                name=f"se{m}_0")
                nc.sync.dma_start(
                    t0[:].rearrange("p a b -> p (a b)"),
                    se_d[:, m * LEN * BL:m * LEN * BL + CH[0] * BL])
                ses[(m, 0)] = t0
            ws_sb = cpool.tile([P2, P2 + 2], bf16)
            nc.gpsimd.dma_start(ws_sb[:], ws_d[:])
            selb = cpool.tile([2, P2], bf16)
            nc.gpsimd.dma_start(selb[:], sb_d[:])
            # (weights ride the gpsimd queue, which is otherwise idle at start)
            w_bf = ws_sb[:, 0:P2]
            sel2 = ws_sb[:, P2:P2 + 2]
            stage = cpool.tile([P2, OUTC], f32)

            st = [None] * M           # per-stream PSUM states
            ys = [None] * M           # per-stream multiply outputs
            ybnd = {}                 # (m, bc) -> boundary-step y tile
            s0c = {}                  # (m, chunk) -> corrected first column
            z_ps = {}                 # bc -> shared [2, 24] z PSUM tile
            zr = cb = cb_sb = None
            mrg_at = {}               # k -> (boundary, stream) merge schedule
            for bc in range(NFB):
                for m in range(M):
                    mrg_at[CST[bc + LAG] - 3 + m] = (bc, m)

            for k in range(LEN):
                c = next(i for i in range(NB) if CST[i] <= k < CUM[i])
                kk = k - CST[c]

                # -- prefetch next chunk (one DMA per stream, staggered) --
                if c + 1 < NB and kk in (2, 3, 4):
                    m = kk - 2
                    Kn = CH[c + 1]
                    tnext = epool.tile([P2, Kn, BL], bf16, tag=f"se{m}",
                                       name=f"se{m}_{c + 1}")
                    off = m * LEN * BL + CST[c + 1] * BL
                    nc.sync.dma_start(
                        tnext[:].rearrange("p a b -> p (a b)"),
                        se_d[:, off:off + Kn * BL])
                    ses[(m, c + 1)] = tnext

                # -- boundary bc event schedule (z at kb+2, feedback later) --
                for bc in range(NB - 1):
                    kb = CUM[bc] - 1
                    if k == kb + 2:
                        zt = zpool.tile([2, M * BL], f32, tag="z",
                                        name=f"z{bc}")
                        z_ps[bc] = zt
                        for m in range(M):
                            ym = ybnd.pop((m, bc))
                            nc.tensor.matmul(
                                zt[:, m * BL:(m + 1) * BL], sel2, ym[:],
                                start=True, stop=True, skip_group_check=True)
                    if k == kb + 3:
                        nc.scalar.activation(
                            stage[0:2, M * BL * bc:M * BL * (bc + 1)],
                            z_ps[bc][:], AF.Copy, bias=0.0)
                    if bc < NFB and k == kb + 4:
                        zr = smpool.tile([2, M * BL], bf16, tag="zr")
                        # bf16 rounding of the applied factor is absorbed by
                        # the telescoping; the host uses the staged f32 z
                        with nc.allow_low_precision(reason="renorm factor"):
                            nc.vector.reciprocal(zr[:], z_ps[bc][:])
                    if bc < NFB and k == kb + 5:
                        cb = cbpool.tile([P2, M * BL], f32, tag="cb")
                        nc.tensor.matmul(
                            cb[:], selb[:], zr[:], start=True, stop=True)
                    if bc < NFB and k == kb + 6:
                        cb_sb = smpool.tile([P2, M * BL], bf16, tag="cbsb")
                        nc.scalar.activation(cb_sb[:], cb[:], AF.Copy, bias=0.0)

                # -- feedback merges (Pool: SBUF-only) for chunk bc+LAG --
                if k in mrg_at:
                    bc, m = mrg_at[k]
                    tgt = bc + LAG
                    t01 = smpool.tile([P2, BL], bf16, tag=f"s0c{m}",
                                      name=f"s0c{m}_{tgt}")
                    nc.gpsimd.tensor_tensor(
                        t01[:], ses[(m, tgt)][:, 0, :],
                        cb_sb[:, m * BL:(m + 1) * BL], AL.mult)
                    s0c[(m, tgt)] = t01

                # -- the three stacked streams: multiply (DVE) + matmul --
                for m in range(M):
                    se = ses[(m, c)]
                    if k == 0:
                        rhs = se[:, 0, :]     # all-ones init: y_0 = s_0
                    else:
                        in1 = (s0c.pop((m, c))[:] if (kk == 0 and (m, c) in s0c)
                               else se[:, kk, :])
                        ym = ypool.tile([P2, BL], bf16, tag=f"y{m}",
                                        name=f"y{m}_{k}")
                        nc.vector.tensor_tensor(ym[:], st[m][:], in1, AL.mult)
                        ys[m] = ym
                        rhs = ym[:]
                        if k == CUM[c] - 1:
                            ybnd[(m, c)] = ym
                    st[m] = spools[m].tile([P2, BL], f32, tag=f"st{m}",
                                           name=f"st{m}_{k}")
                    nc.tensor.matmul(
                        st[m][:], w_bf, rhs, start=True, stop=True)

            # ---- tail: last boundary z, final state, one DMA out ----
            bc = NB - 1
            zt = zpool.tile([2, M * BL], f32, tag="z", name="z_last")
            for m in range(M):
                ym = ybnd.pop((m, bc))
                nc.tensor.matmul(
                    zt[:, m * BL:(m + 1) * BL], sel2, ym[:],
                    start=True, stop=True, skip_group_check=True)
            nc.scalar.activation(
                stage[0:2, M * BL * bc:M * BL * (bc + 1)], zt[:],
                AF.Copy, bias=0.0)
            nc.scalar.activation(
                stage[:, M * BL * NB:OUTC], st[M - 1][:], AF.Copy, bias=0.0)
            nc.sync.dma_start(out_d[:], stage[:])

    nc.compile()
    return nc


def _get_nc():
    if "nc" not in _NC_CACHE:
        _NC_CACHE["nc"] = build_nc()
    return _NC_CACHE["nc"]


def make_in_maps(scores, transitions):
    import ml_dtypes

    bf16 = ml_dtypes.bfloat16
    scores = np.asarray(scores, dtype=np.float64)
    trans = np.asarray(transitions, dtype=np.float64)
    E = np.exp(trans)
    wf = np.zeros((P2, P2 + 2), dtype=np.float64)
    wf[0:L, 0:L] = E
    wf[L:P2, L:P2] = E
    wf[0:L, P2] = 1.0            # sel2 col 0: 1^T over chain 2m
    wf[L:P2, P2 + 1] = 1.0       # sel2 col 1: 1^T over chain 2m+1
    wsel = np.ascontiguousarray(wf.astype(bf16))
    selbf = np.zeros((2, P2), dtype=np.float64)
    selbf[0, 0:L] = 1.0
    selbf[1, L:P2] = 1.0
    selbf = np.ascontiguousarray(selbf.astype(bf16))

    w0 = np.exp(trans[START, :])                 # chain-0 exact init
    in_maps = []
    for cix in range(NCORES):
        sc = scores[cix * BL:(cix + 1) * BL]     # (BL, T, L) f64
        se = np.empty((P2, M * LEN, BL), dtype=np.float64)
        for j in range(S):
            mm, r = j // 2, j % 2
            blk = np.exp(
                sc[:, A0[j]:A0[j] + LEN, :] - DELTA).transpose(2, 1, 0)
            if j == 0:
                blk = blk.copy()
                blk[:, 0, :] *= w0[:, None]
            se[r * L:(r + 1) * L, mm * LEN:(mm + 1) * LEN, :] = blk
        se = np.ascontiguousarray(
            se.reshape(P2, M * LEN * BL).astype(bf16))
        in_maps.append({"se_all": se, "wsel": wsel, "selbf": selbf})
    return in_maps


def combine_outputs(results, scores, gold_target, transitions):
    scores = np.asarray(scores, dtype=np.float64)
    gold = np.asarray(gold_target).reshape(-1)
    trans = np.asarray(transitions, dtype=np.float64)
    tg_energy = (B * trans[0, START] + scores[:, :, 0].sum()
                 + trans[0][gold].sum())
    forscore = 0.0
    for cix in range(NCORES):
        out = np.asarray(results[cix]["stage_out"], dtype=np.float64)
        # z[r, bc, m, b]: row r of stream m at boundary bc
        zs = out[0:2, :M * BL * NB].reshape(2, NB, M, BL)
        logz = np.log(zs)                        # (2, NB, M, BL)

        def lz(j, c):
            return logz[j % 2, c, j // 2]        # (BL,)

        def Pf(j, c):
            v = lz(j, c).copy()
            for cp in range(NB):
                if cp + LAG <= c:
                    v += lz(j, cp)
            return v

        la = np.zeros(BL)
        for j in range(1, S):
            la += Pf(j - 1, NB - 1) - Pf(j, JB)
        wfin = out[L + END, M * BL * NB:OUTC]    # chain S-1 END component (BL,)
        fs_b = la + np.log(wfin) + sum(lz(S - 1, cp) for cp in range(NFB)) \
            + T * DELTA
        forscore += fs_b.sum()
    return np.float32((forscore - tg_energy) / B)


def kernel(scores, gold_target, mask, transitions):
    from concourse.bass_utils import run_bass_kernel_spmd

    nc = _get_nc()
    in_maps = make_in_maps(scores, transitions)
    res = run_bass_kernel_spmd(nc, in_maps, list(range(NCORES)))
    return combine_outputs(res.results, scores, gold_target, transitions)


# revision 10
# speedup vs baseline: 2.1924x; 1.0014x over previous
"""CRF loss kernel for Trainium2 (8 NeuronCores, data-parallel over batch).

Math (faithful to the reference):
  loss = (forscore - tg_energy) / B
  tg_energy = B*trans[0,START] + sum_bt scores[b,t,0] + sum_bt trans[0, gold[b,t]]
    (the reference's torch.gather-on-flattened-(L*L) quirk reduces to row 0;
     computed on the host -- it is pure input-side math)
  forscore = sum_b fs_T[b, END], where fs is the standard CRF forward recurrence
    fs_{t+1}[j] = logsumexp_i(fs_t[i] + scores[t,i] + trans[i,j]), fs_0 = trans[START,:]

Device algorithm, linear space with E = exp(trans) (bf16 matmuls, f32 PSUM):
  w_{t+1} = E^T (w_t * s_t), s_t = exp(scores_t - DELTA) (host-computed, bf16).

The dependent chain w -> y -> w is latency-bound on TRN2 (~414 ns per step:
DVE multiply + PE matmul + semaphores), so the kernel SHORTENS THE CHAIN
instead of adding parallel batch work: products of positive matrices mix --
the state direction forgets its initial condition at ~e^-1.4/step here -- so
time is cut into S=6 segments and each mid-stream segment starts from an
all-ones state with a BURN=8-step burn-in (direction error ~e^-11, far below
bf16 noise).  The unknown per-segment scalars telescope through ratios of
1^T y measured at junctions (chain j's burn-in end coincides with chain
j-1's last step, so both measure the SAME functional of the same time point).
Chain 0 needs no burn-in: its exact init exp(trans[START,:]) is folded into
its first score column on the host.

Per core: 6 chains of LEN=92 steps, stacked 2-per-stream on 96 partitions
(rows 0:48 = chain 2m, rows 48:96 = chain 2m+1), with block-diagonal weights
W = [[E, 0], [0, E]].  3 streams interleave, so each stream's 414 ns step
latency hides inside the other two; per step: one DVE multiply [96,8] and one
PE matmul (the whole local batch of 8 rides in the free dim).  The steady
loop runs NOTHING else on the Vector engine.

No running renormalization is needed: with DELTA=5 the bulk scale 1^T y
drifts within [e^-25, e^1] over a 92-step chain (f64-verified), inside
bf16/f32 range (only negligible tail entries underflow).  The two junction
measurements per chain (z = 1^T y at the burn-in end, step BURN-1, and at the
chain end) are K=96 matmuls against two ones-columns appended to the weight
upload; results are staged to SBUF by the scalar engine and DMA'd out once:
  fs_b = sum_j [log z_end[j-1] - log z_burn[j]] + log w5[END](T) + T*DELTA.

mask is all ones per the problem spec (fill: ones) and is not materialized.
"""

import numpy as np

B, T, L = 64, 512, 48
START, PAD, END = 46, 45, 47
NCORES = 8
BL = B // NCORES          # 8 batch elements per core (all ride the free dim)
P2 = 2 * L                # 96 partitions: two chains per stream
M = 3                     # streams (instruction-level independent chains)
S = 2 * M                 # time segments / chains
BURN = 8                  # burn-in steps for mid-stream chains
LEN = (T + (S - 1) * BURN) // S   # 92 steps per chain
SEG = LEN - BURN          # 84 real steps per mid-stream chain
CH = [BURN, LEN - BURN]   # DMA chunks: [8, 84]
CST = [0, BURN]
DELTA = 5.0
OUTC = 2 * M * BL + BL    # stage cols: 2 z blocks (24 each) + final state
A0 = [0] + [LEN + (j - 1) * SEG - BURN for j in range(1, S)]  # chain starts

_NC_CACHE = {}


def build_nc():
    import concourse.bacc as bacc
    import concourse.mybir as mybir
    import concourse.tile as tile

    f32 = mybir.dt.float32
    bf16 = mybir.dt.bfloat16
    AF = mybir.ActivationFunctionType
    AL = mybir.AluOpType

    nc = bacc.Bacc("TRN2", target_bir_lowering=False, debug=False)

    se_d = nc.dram_tensor("se_all", [P2, M * LEN * BL], bf16, kind="ExternalInput")
    ws_d = nc.dram_tensor("wsel", [P2, P2 + 2], bf16, kind="ExternalInput")
    out_d = nc.dram_tensor("stage_out", [P2, OUTC], f32, kind="ExternalOutput")

    with tile.TileContext(nc) as tc:
        with (
            tc.tile_pool(name="const", bufs=1) as cpool,
            tc.tile_pool(name="sexp", bufs=2) as epool,
            tc.tile_pool(name="ys", bufs=6) as ypool,
            tc.tile_pool(name="st0", bufs=2, space="PSUM") as p0,
            tc.tile_pool(name="st1", bufs=2, space="PSUM") as p1,
            tc.tile_pool(name="st2", bufs=2, space="PSUM") as p2,
            tc.tile_pool(name="zps", bufs=2, space="PSUM") as zpool,
        ):
            spools = [p0, p1, p2]

            # ---- startup: weights + first chunks spread over three DMA
            # queues so their latencies overlap (no activation tables, no
            # device exp -- the host pre-exps into bf16) ----
            ws_sb = cpool.tile([P2, P2 + 2], bf16)
            nc.gpsimd.dma_start(ws_sb[:], ws_d[:])
            ses = {}
            dmaq = [nc.sync, nc.scalar, nc.gpsimd]
            for m in range(M):
                t0 = epool.tile([P2, CH[0], BL], bf16, tag=f"se{m}",
                                name=f"se{m}_0")
                dmaq[m].dma_start(
                    t0[:].rearrange("p a b -> p (a b)"),
                    se_d[:, m * LEN * BL:m * LEN * BL + CH[0] * BL])
                ses[(m, 0)] = t0
            w_bf = ws_sb[:, 0:P2]
            sel2 = ws_sb[:, P2:P2 + 2]
            stage = cpool.tile([P2, OUTC], f32)

            st = [None] * M           # per-stream PSUM states
            ybnd = {}                 # (m, 0|1) -> junction y tiles

            for k in range(LEN):
                c = 0 if k < BURN else 1
                kk = k - CST[c]

                # prefetch the big chunk early, one DMA queue per stream
                if c == 0 and kk in (2, 3, 4):
                    m = kk - 2
                    Kn = CH[1]
                    tnext = epool.tile([P2, Kn, BL], bf16, tag=f"se{m}",
                                       name=f"se{m}_1")
                    off = m * LEN * BL + BURN * BL
                    dmaq[m].dma_start(
                        tnext[:].rearrange("p a b -> p (a b)"),
                        se_d[:, off:off + Kn * BL])
                    ses[(m, 1)] = tnext

                # junction z at the burn-in end: one matmul per stream, two
                # rounds after the measured step (y tiles still live)
                if k == BURN + 1:
                    zt = zpool.tile([2, M * BL], f32, tag="z", name="z_burn")
                    for m in range(M):
                        ym = ybnd[(m, 0)]
                        nc.tensor.matmul(
                            zt[:, m * BL:(m + 1) * BL], sel2, ym[:],
                            start=True, stop=True, skip_group_check=True)
                if k == BURN + 2:
                    nc.scalar.activation(
                        stage[0:2, 0:M * BL], zt[:], AF.Copy, bias=0.0)

                # the three stacked streams: multiply (DVE) + matmul (PE)
                for m in range(M):
                    se = ses[(m, c)]
                    if k == 0:
                        rhs = se[:, 0, :]     # all-ones init: y_0 = s_0
                    else:
                        ym = ypool.tile([P2, BL], bf16, tag=f"y{m}",
                                        name=f"y{m}_{k}")
                        nc.vector.tensor_tensor(
                            ym[:], st[m][:], se[:, kk, :], AL.mult)
                        rhs = ym[:]
                        if k == BURN - 1:
                            ybnd[(m, 0)] = ym
                        elif k == LEN - 1:
                            ybnd[(m, 1)] = ym
                    st[m] = spools[m].tile([P2, BL], f32, tag=f"st{m}",
                                           name=f"st{m}_{k}")
                    nc.tensor.matmul(
                        st[m][:], w_bf, rhs, start=True, stop=True)

            # ---- tail: end-of-chain z, final state, one DMA out ----
            zt2 = zpool.tile([2, M * BL], f32, tag="z", name="z_end")
            for m in range(M):
                ym = ybnd[(m, 1)]
                nc.tensor.matmul(
                    zt2[:, m * BL:(m + 1) * BL], sel2, ym[:],
                    start=True, stop=True, skip_group_check=True)
            nc.scalar.activation(
                stage[0:2, M * BL:2 * M * BL], zt2[:], AF.Copy, bias=0.0)
            nc.scalar.activation(
                stage[:, 2 * M * BL:OUTC], st[M - 1][:], AF.Copy, bias=0.0)
            nc.sync.dma_start(out_d[:], stage[:])

    nc.compile()
    return nc


def _get_nc():
    if "nc" not in _NC_CACHE:
        _NC_CACHE["nc"] = build_nc()
    return _NC_CACHE["nc"]


def make_in_maps(scores, transitions):
    import ml_dtypes

    bf16 = ml_dtypes.bfloat16
    scores = np.asarray(scores, dtype=np.float64)
    trans = np.asarray(transitions, dtype=np.float64)
    E = np.exp(trans)
    wf = np.zeros((P2, P2 + 2), dtype=np.float64)
    wf[0:L, 0:L] = E
    wf[L:P2, L:P2] = E
    wf[0:L, P2] = 1.0            # sel2 col 0: 1^T over chain 2m
    wf[L:P2, P2 + 1] = 1.0       # sel2 col 1: 1^T over chain 2m+1
    wsel = np.ascontiguousarray(wf.astype(bf16))

    w0 = np.exp(trans[START, :])                 # chain-0 exact init
    in_maps = []
    for cix in range(NCORES):
        sc = scores[cix * BL:(cix + 1) * BL]     # (BL, T, L) f64
        se = np.empty((P2, M * LEN, BL), dtype=np.float64)
        for j in range(S):
            mm, r = j // 2, j % 2
            blk = np.exp(
                sc[:, A0[j]:A0[j] + LEN, :] - DELTA).transpose(2, 1, 0)
            if j == 0:
                blk = blk.copy()
                blk[:, 0, :] *= w0[:, None]
            se[r * L:(r + 1) * L, mm * LEN:(mm + 1) * LEN, :] = blk
        se = np.ascontiguousarray(
            se.reshape(P2, M * LEN * BL).astype(bf16))
        in_maps.append({"se_all": se, "wsel": wsel})
    return in_maps


def combine_outputs(results, scores, gold_target, transitions):
    scores = np.asarray(scores, dtype=np.float64)
    gold = np.asarray(gold_target).reshape(-1)
    trans = np.asarray(transitions, dtype=np.float64)
    tg_energy = (B * trans[0, START] + scores[:, :, 0].sum()
                 + trans[0][gold].sum())
    forscore = 0.0
    for cix in range(NCORES):
        out = np.asarray(results[cix]["stage_out"], dtype=np.float64)
        # z[r, 0|1, m, b]: burn-in-end and chain-end measures per stream
        zs = out[0:2, :2 * M * BL].reshape(2, 2, M, BL)
        logz = np.log(zs)

        def lz(j, c):                            # c: 0=burn, 1=end
            return logz[j % 2, c, j // 2]        # (BL,)

        la = np.zeros(BL)
        for j in range(1, S):
            la += lz(j - 1, 1) - lz(j, 0)
        wfin = out[L + END, 2 * M * BL:OUTC]     # chain S-1 END component
        fs_b = la + np.log(wfin) + T * DELTA
        forscore += fs_b.sum()
    return np.float32((forscore - tg_energy) / B)


def kernel(scores, gold_target, mask, transitions):
    from concourse.bass_utils import run_bass_kernel_spmd

    nc = _get_nc()
    in_maps = make_in_maps(scores, transitions)
    res = run_bass_kernel_spmd(nc, in_maps, list(range(NCORES)))
    return combine_outputs(res.results, scores, gold_target, transitions)


# revision 11
# speedup vs baseline: 2.2417x; 1.0225x over previous
"""CRF loss kernel for Trainium2 (8 NeuronCores, data-parallel over batch).

Math (faithful to the reference):
  loss = (forscore - tg_energy) / B
  tg_energy = B*trans[0,START] + sum_bt scores[b,t,0] + sum_bt trans[0, gold[b,t]]
    (the reference's torch.gather-on-flattened-(L*L) quirk reduces to row 0;
     computed on the host -- it is pure input-side math)
  forscore = sum_b fs_T[b, END], where fs is the standard CRF forward recurrence
    fs_{t+1}[j] = logsumexp_i(fs_t[i] + scores[t,i] + trans[i,j]), fs_0 = trans[START,:]

Device algorithm, linear space with E = exp(trans) (bf16 matmuls, f32 PSUM):
  w_{t+1} = E^T (w_t * s_t), s_t = exp(scores_t - DELTA) (host-computed, bf16).

The dependent chain w -> y -> w is latency-bound on TRN2 (~414 ns per step:
DVE multiply + PE matmul + semaphores), so the kernel SHORTENS THE CHAIN
instead of adding parallel batch work: products of positive matrices mix --
the state direction forgets its initial condition at ~e^-1.4/step here -- so
time is cut into S=6 segments and each mid-stream segment starts from an
all-ones state with a BURN=8-step burn-in (direction error ~e^-11, far below
bf16 noise).  The unknown per-segment scalars telescope through ratios of
1^T y measured at junctions (chain j's burn-in end coincides with chain
j-1's last step, so both measure the SAME functional of the same time point).
Chain 0 needs no burn-in: its exact init exp(trans[START,:]) is folded into
its first score column on the host.

Per core: 6 chains of LEN=92 steps, stacked 2-per-stream on 96 partitions
(rows 0:48 = chain 2m, rows 48:96 = chain 2m+1), with block-diagonal weights
W = [[E, 0], [0, E]].  3 streams interleave, so each stream's 414 ns step
latency hides inside the other two; per step: one DVE multiply [96,8] and one
PE matmul (the whole local batch of 8 rides in the free dim).  The steady
loop runs NOTHING else on the Vector engine.

No running renormalization is needed: with DELTA=5 the bulk scale 1^T y
drifts within [e^-25, e^1] over a 92-step chain (f64-verified), inside
bf16/f32 range (only negligible tail entries underflow).  The two junction
measurements per chain (z = 1^T y at the burn-in end, step BURN-1, and at the
chain end) are K=96 matmuls against two ones-columns appended to the weight
upload; results are staged to SBUF by the scalar engine and DMA'd out once:
  fs_b = sum_j [log z_end[j-1] - log z_burn[j]] + log w5[END](T) + T*DELTA.

mask is all ones per the problem spec (fill: ones) and is not materialized.
"""

import numpy as np

B, T, L = 64, 512, 48
START, PAD, END = 46, 45, 47
NCORES = 8
BL = B // NCORES          # 8 batch elements per core (all ride the free dim)
P2 = 2 * L                # 96 partitions: two chains per stream
M = 3                     # streams (instruction-level independent chains)
S = 2 * M                 # time segments / chains
BURN = 8                  # burn-in steps for mid-stream chains
LEN = (T + (S - 1) * BURN) // S   # 92 steps per chain
SEG = LEN - BURN          # 84 real steps per mid-stream chain
CH = [BURN, LEN - BURN]   # DMA chunks: [8, 84]
CST = [0, BURN]
DELTA = 5.0
OUTC = 2 * M * BL + BL    # stage cols: 2 z blocks (24 each) + final state
A0 = [0] + [LEN + (j - 1) * SEG - BURN for j in range(1, S)]  # chain starts

_NC_CACHE = {}


def build_nc():
    import concourse.bacc as bacc
    import concourse.mybir as mybir
    import concourse.tile as tile

    f32 = mybir.dt.float32
    bf16 = mybir.dt.bfloat16
    AF = mybir.ActivationFunctionType
    AL = mybir.AluOpType

    nc = bacc.Bacc("TRN2", target_bir_lowering=False, debug=False)

    se_d = nc.dram_tensor("se_all", [P2, M * LEN * BL], bf16, kind="ExternalInput")
    ws_d = nc.dram_tensor("wsel", [P2, P2 + 2], bf16, kind="ExternalInput")
    out_d = nc.dram_tensor("stage_out", [P2, OUTC], f32, kind="ExternalOutput")

    with tile.TileContext(nc) as tc:
        with (
            tc.tile_pool(name="const", bufs=1) as cpool,
            tc.tile_pool(name="sexp", bufs=2) as epool,
            tc.tile_pool(name="ys", bufs=6) as ypool,
            tc.tile_pool(name="st0", bufs=2, space="PSUM") as p0,
            tc.tile_pool(name="st1", bufs=2, space="PSUM") as p1,
            tc.tile_pool(name="st2", bufs=2, space="PSUM") as p2,
            tc.tile_pool(name="zps", bufs=2, space="PSUM") as zpool,
        ):
            spools = [p0, p1, p2]

            # ---- startup: weights + first chunks spread over three DMA
            # queues so their latencies overlap (no activation tables, no
            # device exp -- the host pre-exps into bf16) ----
            # weights gate LDWEIGHTS for every matmul: first on the sync
            # queue; stream-0's first chunk in parallel on the scalar queue
            ws_sb = cpool.tile([P2, P2 + 2], bf16)
            nc.sync.dma_start(ws_sb[:], ws_d[:])
            ses = {}
            dmaq = [nc.scalar, nc.sync, nc.gpsimd]
            for m in range(M):
                t0 = epool.tile([P2, CH[0], BL], bf16, tag=f"se{m}",
                                name=f"se{m}_0")
                dmaq[m].dma_start(
                    t0[:].rearrange("p a b -> p (a b)"),
                    se_d[:, m * LEN * BL:m * LEN * BL + CH[0] * BL])
                ses[(m, 0)] = t0
            w_bf = ws_sb[:, 0:P2]
            sel2 = ws_sb[:, P2:P2 + 2]
            stage = cpool.tile([P2, OUTC], f32)

            st = [None] * M           # per-stream PSUM states
            ybnd = {}                 # (m, 0|1) -> junction y tiles

            for k in range(LEN):
                c = 0 if k < BURN else 1
                kk = k - CST[c]

                # prefetch the big chunk early, one DMA queue per stream
                if c == 0 and kk in (2, 3, 4):
                    m = kk - 2
                    Kn = CH[1]
                    tnext = epool.tile([P2, Kn, BL], bf16, tag=f"se{m}",
                                       name=f"se{m}_1")
                    off = m * LEN * BL + BURN * BL
                    dmaq[m].dma_start(
                        tnext[:].rearrange("p a b -> p (a b)"),
                        se_d[:, off:off + Kn * BL])
                    ses[(m, 1)] = tnext

                # junction z at the burn-in end: one matmul per round
                # starting two rounds after the measured step (y still live)
                if BURN + 1 <= k < BURN + 1 + M:
                    m = k - BURN - 1
                    if m == 0:
                        zt = zpool.tile([2, M * BL], f32, tag="z",
                                        name="z_burn")
                    nc.tensor.matmul(
                        zt[:, m * BL:(m + 1) * BL], sel2, ybnd[(m, 0)][:],
                        start=True, stop=True, skip_group_check=True)
                if k == BURN + 1 + M:
                    nc.scalar.activation(
                        stage[0:2, 0:M * BL], zt[:], AF.Copy, bias=0.0)

                # the three stacked streams: multiply (DVE) + matmul (PE)
                for m in range(M):
                    se = ses[(m, c)]
                    if k == 0:
                        rhs = se[:, 0, :]     # all-ones init: y_0 = s_0
                    else:
                        ym = ypool.tile([P2, BL], bf16, tag=f"y{m}",
                                        name=f"y{m}_{k}")
                        nc.vector.tensor_tensor(
                            ym[:], st[m][:], se[:, kk, :], AL.mult)
                        rhs = ym[:]
                        if k == BURN - 1:
                            ybnd[(m, 0)] = ym
                        elif k == LEN - 1:
                            ybnd[(m, 1)] = ym
                    st[m] = spools[m].tile([P2, BL], f32, tag=f"st{m}",
                                           name=f"st{m}_{k}")
                    nc.tensor.matmul(
                        st[m][:], w_bf, rhs, start=True, stop=True)

            # ---- tail: end-of-chain z, final state, one DMA out ----
            zt2 = zpool.tile([2, M * BL], f32, tag="z", name="z_end")
            for m in range(M):
                ym = ybnd[(m, 1)]
                nc.tensor.matmul(
                    zt2[:, m * BL:(m + 1) * BL], sel2, ym[:],
                    start=True, stop=True, skip_group_check=True)
            nc.scalar.activation(
                stage[0:2, M * BL:2 * M * BL], zt2[:], AF.Copy, bias=0.0)
            nc.scalar.activation(
                stage[:, 2 * M * BL:OUTC], st[M - 1][:], AF.Copy, bias=0.0)
            nc.sync.dma_start(out_d[:], stage[:])

    nc.compile()
    return nc


def _get_nc():
    if "nc" not in _NC_CACHE:
        _NC_CACHE["nc"] = build_nc()
    return _NC_CACHE["nc"]


def make_in_maps(scores, transitions):
    import ml_dtypes

    bf16 = ml_dtypes.bfloat16
    scores = np.asarray(scores, dtype=np.float64)
    trans = np.asarray(transitions, dtype=np.float64)
    E = np.exp(trans)
    wf = np.zeros((P2, P2 + 2), dtype=np.float64)
    wf[0:L, 0:L] = E
    wf[L:P2, L:P2] = E
    wf[0:L, P2] = 1.0            # sel2 col 0: 1^T over chain 2m
    wf[L:P2, P2 + 1] = 1.0       # sel2 col 1: 1^T over chain 2m+1
    wsel = np.ascontiguousarray(wf.astype(bf16))

    w0 = np.exp(trans[START, :])                 # chain-0 exact init
    in_maps = []
    for cix in range(NCORES):
        sc = scores[cix * BL:(cix + 1) * BL]     # (BL, T, L) f64
        se = np.empty((P2, M * LEN, BL), dtype=np.float64)
        for j in range(S):
            mm, r = j // 2, j % 2
            blk = np.exp(
                sc[:, A0[j]:A0[j] + LEN, :] - DELTA).transpose(2, 1, 0)
            if j == 0:
                blk = blk.copy()
                blk[:, 0, :] *= w0[:, None]
            se[r * L:(r + 1) * L, mm * LEN:(mm + 1) * LEN, :] = blk
        se = np.ascontiguousarray(
            se.reshape(P2, M * LEN * BL).astype(bf16))
        in_maps.append({"se_all": se, "wsel": wsel})
    return in_maps


def combine_outputs(results, scores, gold_target, transitions):
    scores = np.asarray(scores, dtype=np.float64)
    gold = np.asarray(gold_target).reshape(-1)
    trans = np.asarray(transitions, dtype=np.float64)
    tg_energy = (B * trans[0, START] + scores[:, :, 0].sum()
                 + trans[0][gold].sum())
    forscore = 0.0
    for cix in range(NCORES):
        out = np.asarray(results[cix]["stage_out"], dtype=np.float64)
        # z[r, 0|1, m, b]: burn-in-end and chain-end measures per stream
        zs = out[0:2, :2 * M * BL].reshape(2, 2, M, BL)
        logz = np.log(zs)

        def lz(j, c):                            # c: 0=burn, 1=end
            return logz[j % 2, c, j // 2]        # (BL,)

        la = np.zeros(BL)
        for j in range(1, S):
            la += lz(j - 1, 1) - lz(j, 0)
        wfin = out[L + END, 2 * M * BL:OUTC]     # chain S-1 END component
        fs_b = la + np.log(wfin) + T * DELTA
        forscore += fs_b.sum()
    return np.float32((forscore - tg_energy) / B)


def kernel(scores, gold_target, mask, transitions):
    from concourse.bass_utils import run_bass_kernel_spmd

    nc = _get_nc()
    in_maps = make_in_maps(scores, transitions)
    res = run_bass_kernel_spmd(nc, in_maps, list(range(NCORES)))
    return combine_outputs(res.results, scores, gold_target, transitions)


# revision 12
# speedup vs baseline: 3.4723x; 1.5490x over previous
"""CRF loss kernel for Trainium2 (8 NeuronCores, time-sharded).

Math (faithful to the reference):
  loss = (forscore - tg_energy) / B
  tg_energy = B*trans[0,START] + sum_bt scores[b,t,0] + sum_bt trans[0, gold[b,t]]
    (the reference's torch.gather-on-flattened-(L*L) quirk reduces to row 0;
     computed on the host -- it is pure input-side math)
  forscore = sum_b fs_T[b, END], where fs is the standard CRF forward recurrence
    fs_{t+1}[j] = logsumexp_i(fs_t[i] + scores[t,i] + trans[i,j]), fs_0 = trans[START,:]

Device algorithm, linear space with E = exp(trans) (bf16 matmuls, f32 PSUM):
  w_{t+1} = E^T (w_t * s_t), s_t = exp(scores_t - DELTA) (host-computed, bf16).

The dependent chain w -> y -> w is bound by per-instruction overhead and
latency on TRN2 (~130-190 ns per engine op, ~500 ns per dependent step), so
the kernel amortizes each instruction over the ENTIRE batch (64 wide) and
shortens the chain by sharding TIME -- not batch -- across cores: products of
positive matrices mix (the state direction forgets its initial condition at
~e^-1.4/step here), so time is cut into 16 segments of 32 steps; each core
runs 2 chains of LEN=40 steps ([48 tags x 64 batch] tiles), where chain
j >= 1 starts from an all-ones state 8 steps early (burn-in; direction error
~e^-11, far below bf16 noise).  Chain 0 is exact from t=0: its init
exp(trans[START,:]) is folded into its first score column on the host (its
last 8 steps are padding).  Cores never communicate; the host telescopes the
unknown per-segment scalars through ratios of 1^T w at junctions, where
chain j's state at burn-in end (step 8, time 32j) coincides in time with
chain j-1's final state (step 40; step 32 for chain 0).

Per core, per step: one DVE multiply [48,64] and one PE matmul per chain,
with the two chains phase-interleaved so each hides the other's
DVE->PE->DVE latency.  No renormalization is needed: the bulk scale drifts
within e^{+-13} over a 40-step chain with DELTA=5 (f64-verified), far inside
f32/bf16 range.  Full state snapshots at steps 8, 32, and 40 are staged to
SBUF by the (otherwise idle) scalar engine and DMA'd out once; the host does
the junction ratios, the END-component pick, and tg_energy in f64:
  fs_b = sum_j [log 1^T w_out_{j-1} - log 1^T w_in_j] + log w15[END] + T*DELTA.

mask is all ones per the problem spec (fill: ones) and is not materialized.
"""

import numpy as np

B, T, L = 64, 512, 48
START, PAD, END = 46, 45, 47
NCORES = 8
NCHAIN = 2 * NCORES       # 16 time-segment chains, 2 per core
SEG = T // NCHAIN         # 32 real steps per chain
BURN = 8                  # burn-in steps (chains 1..15)
LEN = SEG + BURN          # 40 steps per chain
CH0 = 8                   # startup DMA chunk (rest arrives in one big chunk)
DELTA = 5.0
SNAPS = (BURN, 32, LEN)   # staged state snapshots (after that many steps)
OUTC = 6 * B              # stage cols: 3 snapshots x 2 chains x 64 batch
A0 = [0] + [SEG * j - BURN for j in range(1, NCHAIN)]  # chain stream starts

_NC_CACHE = {}


def build_nc():
    import concourse.bacc as bacc
    import concourse.mybir as mybir
    import concourse.tile as tile

    f32 = mybir.dt.float32
    bf16 = mybir.dt.bfloat16
    AF = mybir.ActivationFunctionType
    AL = mybir.AluOpType

    nc = bacc.Bacc("TRN2", target_bir_lowering=False, debug=False)

    se_d = nc.dram_tensor("se_all", [L, 2 * LEN * B], bf16, kind="ExternalInput")
    e_d = nc.dram_tensor("e_mat", [L, L], bf16, kind="ExternalInput")
    out_d = nc.dram_tensor("stage_out", [L, OUTC], f32, kind="ExternalOutput")

    with tile.TileContext(nc) as tc:
        with (
            tc.tile_pool(name="const", bufs=1) as cpool,
            tc.tile_pool(name="sexp", bufs=2) as epool,
            tc.tile_pool(name="ys", bufs=4) as ypool,
            tc.tile_pool(name="st0", bufs=3, space="PSUM") as p0,
            tc.tile_pool(name="st1", bufs=3, space="PSUM") as p1,
        ):
            spools = [p0, p1]

            # ---- startup: weights gate every matmul's LDWEIGHTS -> first on
            # the sync queue; the two chains' first chunks in parallel on the
            # sync/scalar queues, big chunks behind them ----
            e_sb = cpool.tile([L, L], bf16)
            nc.sync.dma_start(e_sb[:], e_d[:])
            ses = {}
            dmaq = [nc.sync, nc.scalar]
            for p in range(2):
                t0 = epool.tile([L, CH0, B], bf16, tag=f"se{p}",
                                name=f"se{p}_0")
                dmaq[p].dma_start(
                    t0[:].rearrange("p a b -> p (a b)"),
                    se_d[:, p * LEN * B:p * LEN * B + CH0 * B])
                ses[(p, 0)] = t0
            for p in range(2):
                t1 = epool.tile([L, LEN - CH0, B], bf16, tag=f"se{p}",
                                name=f"se{p}_1")
                dmaq[1 - p].dma_start(
                    t1[:].rearrange("p a b -> p (a b)"),
                    se_d[:, p * LEN * B + CH0 * B:(p + 1) * LEN * B])
                ses[(p, 1)] = t1
            stage = cpool.tile([L, OUTC], f32)

            st = [None] * 2           # per-chain PSUM states [48, 64]
            snap = {}                 # (p, step) -> state tile to stage

            for k in range(LEN):
                # stage snapshots one round after they were produced (state
                # pools hold 3 bufs, so the tile survives until round k+3)
                for p in range(2):
                    if (p, k) in snap:
                        si = SNAPS.index(k)
                        nc.scalar.activation(
                            stage[:, (3 * p + si) * B:(3 * p + si + 1) * B],
                            snap.pop((p, k))[:], AF.Copy, bias=0.0)

                c = 0 if k < CH0 else 1
                kk = k - (0 if k < CH0 else CH0)
                for p in range(2):
                    se = ses[(p, c)]
                    if k == 0:
                        rhs = se[:, 0, :]     # all-ones init: y_0 = s_0
                    else:
                        ym = ypool.tile([L, B], bf16, tag=f"y{p}",
                                        name=f"y{p}_{k}")
                        nc.vector.tensor_tensor(
                            ym[:], st[p][:], se[:, kk, :], AL.mult)
                        rhs = ym[:]
                    st[p] = spools[p].tile([L, B], f32, tag=f"st{p}",
                                           name=f"st{p}_{k}")
                    nc.tensor.matmul(
                        st[p][:], e_sb[:], rhs, start=True, stop=True)
                    if k + 1 in SNAPS:
                        snap[(p, k + 1)] = st[p]

            # ---- tail: final snapshots, one DMA out ----
            for p in range(2):
                nc.scalar.activation(
                    stage[:, (3 * p + 2) * B:(3 * p + 3) * B],
                    snap.pop((p, LEN))[:], AF.Copy, bias=0.0)
            nc.sync.dma_start(out_d[:], stage[:])

    nc.compile()
    return nc


def _get_nc():
    if "nc" not in _NC_CACHE:
        _NC_CACHE["nc"] = build_nc()
    return _NC_CACHE["nc"]


def make_in_maps(scores, transitions):
    import ml_dtypes

    bf16 = ml_dtypes.bfloat16
    scores = np.asarray(scores, dtype=np.float64)
    trans = np.asarray(transitions, dtype=np.float64)
    E = np.ascontiguousarray(np.exp(trans).astype(bf16))
    w0 = np.exp(trans[START, :])                 # chain-0 exact init
    in_maps = []
    for cix in range(NCORES):
        se = np.empty((L, 2, LEN, B), dtype=np.float64)
        for p in range(2):
            g = 2 * cix + p
            blk = np.exp(
                scores[:, A0[g]:A0[g] + LEN, :] - DELTA).transpose(2, 1, 0)
            if g == 0:
                blk = blk.copy()
                blk[:, 0, :] *= w0[:, None]
            se[:, p] = blk
        se = np.ascontiguousarray(se.reshape(L, 2 * LEN * B).astype(bf16))
        in_maps.append({"se_all": se, "e_mat": E})
    return in_maps


def combine_outputs(results, scores, gold_target, transitions):
    scores = np.asarray(scores, dtype=np.float64)
    gold = np.asarray(gold_target).reshape(-1)
    trans = np.asarray(transitions, dtype=np.float64)
    tg_energy = (B * trans[0, START] + scores[:, :, 0].sum()
                 + trans[0][gold].sum())

    # snaps[g][si]: chain g's state after SNAPS[si] steps, shape (L, B)
    snaps = {}
    for cix in range(NCORES):
        out = np.asarray(results[cix]["stage_out"], dtype=np.float64)
        for p in range(2):
            snaps[2 * cix + p] = [
                out[:, (3 * p + si) * B:(3 * p + si + 1) * B]
                for si in range(3)]

    la = np.zeros(B)
    for g in range(1, NCHAIN):
        # chain g-1's state at time SEG*g: step 32 for chain 0, else step 40
        out_prev = snaps[g - 1][1 if g == 1 else 2]
        in_cur = snaps[g][0]
        la += np.log(out_prev.sum(0)) - np.log(in_cur.sum(0))
    wfin = snaps[NCHAIN - 1][2]
    fs_b = la + np.log(wfin[END, :]) + T * DELTA
    forscore = fs_b.sum()
    return np.float32((forscore - tg_energy) / B)


def kernel(scores, gold_target, mask, transitions):
    from concourse.bass_utils import run_bass_kernel_spmd

    nc = _get_nc()
    in_maps = make_in_maps(scores, transitions)
    res = run_bass_kernel_spmd(nc, in_maps, list(range(NCORES)))
    return combine_outputs(res.results, scores, gold_target, transitions)
